# revision 24
# baseline (speedup 1.0000x reference)
"""VQ codebook nearest-centroid kernel for Trainium2 (8 NeuronCores).

Problem: features (65536, 1, 256) f32, mu (1, 1024, 256) f32.
Returns (distance (65536,) f32, index (65536,) int32, maxDist (1,) f32)
matching:
    d2   = ||f||^2 - 2 f.mu + ||mu||^2      (N, K)
    dist = sqrt(max(d2, 0))
    distance = min_k dist ; index = argmin_k dist ; maxDist = max_n distance

Sharding: data-parallel over N; each of the 8 cores handles 8192 rows,
centroids replicated. maxDist folded on host (exact max of gathered mins).

Matmul precision/speed: fp32/f32r matmuls run at 4 cycles/column on this
hardware, so 2 f.mu is computed with a 3-pass fp16 hi/lo split at 1
cycle/column:  f = fh + fl, mu = mh + ml (fp16 each, products exact in the
f32 PSUM accumulate) and  2 f.mu ~= 2[fh.mh + fl.mh + fh.ml]  (dropped
fl.ml term ~1e-6 relative).  argmax_k u over u = 2 f.mu - c_sq equals
argmin_k d2 exactly (per-row monotone shift); ties resolve to the first
index like jnp.argmin.

Per-core device program, per 128-row tile (64 tiles):
  PE  : 12 matmuls (fp16, 512 cols) accumulating 2 f.mu into PSUM [128,1024]
  ACT : copy PSUM -> SBUF
  GPS : u = copy - c_sq (broadcast row, tensor_sub)
  DVE : max (top-8) + max_index -> first occurrence of max u = argmin d2
Tail: one GPSIMD sub (d2min = f_sq - mx) + one ACT Sqrt over all 64 tiles,
then two contiguous DMAs out (partition-major; host transposes).

This container's walrus build only supports ONE sync wait per instruction
and rejects the InstISA encodings concourse emits for
EVENT_SEMAPHORE_RANGE_CLEAR and TENSOR_TENSOR_REDUCE ("ISA wrong length").
Workarounds: native per-sem clears (sem_clear patch), native-only compute
instructions, and a post-pass that hoists extra sync waits onto standalone
EventSemaphore instructions on the same engine.
"""

from contextlib import ExitStack

import numpy as np

import concourse.bass as bass
import concourse.mybir as mybir
import concourse.tile as tile
from concourse.bass_utils import run_bass_kernel_spmd


def _sem_clear_native(self, sems):
    """Replace the EVENT_SEMAPHORE_RANGE_CLEAR InstISA (rejected by this
    container's walrus with "ISA wrong length") with per-semaphore native
    sem-wr-imm EventSemaphore instructions, spread round-robin across the
    engine queues so the clears run in parallel between the two all-engine
    barriers; only sems actually referenced by the program are cleared."""
    nums = list(sems) if isinstance(sems, range) else [sems.num]
    used = set()
    for f in self.bass.m.functions:
        for bb in f.blocks:
            for inst in bb.instructions:
                si = inst.sync_info
                if si is None:
                    continue
                for w in si.on_wait:
                    used.add(w.id)
                for u in si.on_update:
                    used.add(u.id)
    nums = [n for n in nums if n in used]
    b = self.bass
    engines = [b.gpsimd, b.vector, b.scalar, b.tensor, b.sync]
    last = None
    for idx, n in enumerate(nums):
        si = mybir.SyncInfo(
            on_wait=[],
            on_update=[mybir.SyncUpdate(
                sync_type="semaphore", id=n,
                update_mode="sem-wr-imm", update_value=0,
            )],
        )
        eng = engines[idx % len(engines)]
        last = eng.add_instruction(mybir.InstEventSemaphore(
            name=f"I-{b.next_id()}", sync_info=si,
        ))
    return last


bass.BassGpSimd.sem_clear = _sem_clear_native


def split_multi_waits(nc):
    """This walrus allows a single sync wait per instruction; hoist all but
    the last wait of each instruction onto fresh same-engine EventSemaphore
    instructions placed immediately before it."""
    n_split = 0
    for f in nc.m.functions:
        for bb in f.blocks:
            new = []
            for inst in bb.instructions:
                si = inst.sync_info
                if si is not None and len(si.on_wait) > 1:
                    waits = list(si.on_wait)
                    for w in waits[:-1]:
                        new.append(mybir.InstEventSemaphore(
                            name=f"I-{nc.next_id()}",
                            engine=inst.engine,
                            sync_info=mybir.SyncInfo(on_wait=[w], on_update=[]),
                        ))
                        n_split += 1
                    inst.sync_info = mybir.SyncInfo(
                        on_wait=[waits[-1]], on_update=list(si.on_update),
                    )
                new.append(inst)
            bb.instructions = new
    return n_split


N, K, D = 65536, 1024, 256
NCORES = 8
NSHARD = N // NCORES          # 8192 rows per core
P = 128                       # partitions
NTILES = NSHARD // P          # 64 row-tiles per core
DCH = D // P                  # 2 contraction chunks
MMW = 512                     # matmul moving width (psum bank limit)
KCH = K // MMW
GROUP = 4                     # row-tiles per feature DMA
PS_BUFS = 4

MM_DT = mybir.dt.float16      # matmul operand dtype, 1 cycle/column
F32 = mybir.dt.float32


def build_program(split_waits=True):
    """Build the single-core Bass/Tile program (same program on all 8 cores)."""
    nc = bass.Bass("TRN2", target_bir_lowering=False, debug=False)

    fhi = nc.dram_tensor("fhi", (DCH, P, NSHARD), MM_DT, kind="ExternalInput").ap()
    flo = nc.dram_tensor("flo", (DCH, P, NSHARD), MM_DT, kind="ExternalInput").ap()
    muhi = nc.dram_tensor("muhi", (DCH, P, K), MM_DT, kind="ExternalInput").ap()
    mulo = nc.dram_tensor("mulo", (DCH, P, K), MM_DT, kind="ExternalInput").ap()
    csqb = nc.dram_tensor("csqb", (P, K), F32, kind="ExternalInput").ap()
    fsq = nc.dram_tensor("fsq", (NTILES, P), F32, kind="ExternalInput").ap()
    dist = nc.dram_tensor("dist", (P, NTILES), F32, kind="ExternalOutput").ap()
    idx = nc.dram_tensor("idx", (P, NTILES), mybir.dt.uint32, kind="ExternalOutput").ap()

    with tile.TileContext(nc) as tc, ExitStack() as ctx:
        const = ctx.enter_context(tc.tile_pool(name="const", bufs=1))
        ftp = ctx.enter_context(tc.tile_pool(name="ft", bufs=4))
        psp = ctx.enter_context(tc.tile_pool(name="ps", bufs=PS_BUFS, space="PSUM"))
        up = ctx.enter_context(tc.tile_pool(name="u", bufs=4))
        urp = ctx.enter_context(tc.tile_pool(name="ur", bufs=4))

        # Resident constants
        mh_sb = const.tile([P, DCH, K], MM_DT, tag="mh")
        nc.sync.dma_start(mh_sb[:], muhi.rearrange("d p k -> p d k"))
        ml_sb = const.tile([P, DCH, K], MM_DT, tag="ml")
        nc.sync.dma_start(ml_sb[:], mulo.rearrange("d p k -> p d k"))
        csq_sb = const.tile([P, K], F32, tag="csq")
        nc.sync.dma_start(csq_sb[:], csqb)
        fsq_sb = const.tile([P, NTILES], F32, tag="fsq")
        nc.sync.dma_start(fsq_sb[:], fsq.rearrange("t p -> p t"))
        dist_acc = const.tile([P, NTILES], F32, tag="dacc")
        d2min = const.tile([P, NTILES], F32, tag="d2min")
        mx_acc = const.tile([P, NTILES, 8], F32, tag="mxacc")
        idx_acc = const.tile([P, NTILES, 8], mybir.dt.uint32, tag="iacc")

        for g in range(NTILES // GROUP):
            gsl = slice(g * GROUP * P, (g + 1) * GROUP * P)
            fh_sb = ftp.tile([P, DCH, GROUP * P], MM_DT, tag="fh", name="fh_sb")
            nc.sync.dma_start(fh_sb[:], fhi[:, :, gsl].rearrange("d p n -> p d n"))
            fl_sb = ftp.tile([P, DCH, GROUP * P], MM_DT, tag="fl", name="fl_sb")
            nc.sync.dma_start(fl_sb[:], flo[:, :, gsl].rearrange("d p n -> p d n"))
            for j in range(GROUP):
                i = g * GROUP + j
                jsl = slice(j * P, (j + 1) * P)
                ps = psp.tile([P, K], F32)
                for kc in range(KCH):
                    sl = slice(kc * MMW, (kc + 1) * MMW)
                    mms = [
                        (fh_sb[:, 0, jsl], mh_sb[:, 0, sl]),
                        (fh_sb[:, 0, jsl], ml_sb[:, 0, sl]),
                        (fh_sb[:, 1, jsl], mh_sb[:, 1, sl]),
                        (fh_sb[:, 1, jsl], ml_sb[:, 1, sl]),
                        (fl_sb[:, 0, jsl], mh_sb[:, 0, sl]),
                        (fl_sb[:, 1, jsl], mh_sb[:, 1, sl]),
                    ]
                    for m, (lh, rh) in enumerate(mms):
                        nc.tensor.matmul(
                            ps[:, sl],
                            lhsT=lh,
                            rhs=rh,
                            start=(m == 0),
                            stop=(m == len(mms) - 1),
                        )
                ur = urp.tile([P, K], F32, tag="ur", name="ur")
                nc.scalar.copy(ur[:], ps[:])
                u = up.tile([P, K], F32)
                nc.gpsimd.tensor_sub(u[:], ur[:], csq_sb[:])
                nc.vector.max(mx_acc[:, i, :], u[:])
                nc.vector.max_index(idx_acc[:, i, :], mx_acc[:, i, :], u[:])

        # d2min = f_sq - max_k u, distance = sqrt(d2min): batched over all tiles
        nc.gpsimd.tensor_sub(d2min[:], fsq_sb[:], mx_acc[:, :, 0])
        nc.scalar.activation(dist_acc[:], d2min[:],
                             mybir.ActivationFunctionType.Sqrt)
        nc.sync.dma_start(dist, dist_acc[:])
        nc.sync.dma_start(idx, idx_acc[:, :, 0])

    if split_waits:
        split_multi_waits(nc)
    return nc


def _split16(x):
    hi = x.astype(np.float16)
    lo = (x - hi.astype(np.float32)).astype(np.float16)
    return hi, lo


def host_inputs(features, mu):
    """Host-side prep: transpose/shard features, fp16 hi/lo split, norms."""
    f = np.ascontiguousarray(features.reshape(N, D), dtype=np.float32)
    c = np.ascontiguousarray(mu.reshape(K, D), dtype=np.float32)
    fT = np.ascontiguousarray(f.T)                       # (D, N)
    muT2 = np.ascontiguousarray((2.0 * c).T)             # (D, K)
    csq = np.einsum("kd,kd->k", c, c).astype(np.float32)  # (K,)
    fsq = np.einsum("nd,nd->n", f, f).astype(np.float32)  # (N,)
    fT_hi, fT_lo = _split16(fT)
    mu_hi, mu_lo = _split16(muT2)
    csqb = np.ascontiguousarray(np.broadcast_to(csq, (P, K)))
    in_maps = []
    for core in range(NCORES):
        sl = slice(core * NSHARD, (core + 1) * NSHARD)
        in_maps.append({
            "fhi": np.ascontiguousarray(fT_hi[:, sl]).reshape(DCH, P, NSHARD),
            "flo": np.ascontiguousarray(fT_lo[:, sl]).reshape(DCH, P, NSHARD),
            "muhi": mu_hi.reshape(DCH, P, K),
            "mulo": mu_lo.reshape(DCH, P, K),
            "csqb": csqb,
            "fsq": np.ascontiguousarray(fsq[sl]).reshape(NTILES, P),
        })
    return in_maps


_PROGRAM = None


def _program():
    global _PROGRAM
    if _PROGRAM is None:
        _PROGRAM = build_program()
    return _PROGRAM


def kernel(features, mu, _trace=False):
    features = np.asarray(features)
    mu = np.asarray(mu)
    nc = _program()
    in_maps = host_inputs(features, mu)
    res = run_bass_kernel_spmd(nc, in_maps, list(range(NCORES)), trace=_trace)
    distance = np.concatenate(
        [r["dist"].T.reshape(NSHARD) for r in res.results])
    index = np.concatenate(
        [r["idx"].T.reshape(NSHARD).astype(np.int32) for r in res.results]
    )
    max_dist = np.max(distance).reshape(1).astype(np.float32)
    if _trace:
        return (distance, index, max_dist), res
    return (distance, index, max_dist)


# revision 26
# speedup vs baseline: 1.0534x; 1.0534x over previous
"""VQ codebook nearest-centroid kernel for Trainium2 (8 NeuronCores).

Problem: features (65536, 1, 256) f32, mu (1, 1024, 256) f32.
Returns (distance (65536,) f32, index (65536,) int32, maxDist (1,) f32)
matching:
    d2   = ||f||^2 - 2 f.mu + ||mu||^2      (N, K)
    dist = sqrt(max(d2, 0))
    distance = min_k dist ; index = argmin_k dist ; maxDist = max_n distance

Sharding: data-parallel over N; each of the 8 cores handles 8192 rows,
centroids replicated. maxDist folded on host (exact max of gathered mins).

Matmul precision/speed: fp32/f32r matmuls run at 4 cycles/column on this
hardware, so 2 f.mu is computed with a 3-pass fp16 hi/lo split at 1
cycle/column:  f = fh + fl, mu = mh + ml (fp16 each, products exact in the
f32 PSUM accumulate) and  2 f.mu ~= 2[fh.mh + fl.mh + fh.ml]  (dropped
fl.ml term ~1e-6 relative).  argmax_k u over u = 2 f.mu - c_sq equals
argmin_k d2 exactly (per-row monotone shift); ties resolve to the first
index like jnp.argmin.

Per-core device program, per 128-row tile (64 tiles):
  PE  : 12 matmuls (fp16, 512 cols) accumulating 2 f.mu into PSUM [128,1024]
  ACT : copy PSUM -> SBUF
  GPS : u = copy - c_sq (broadcast row, tensor_sub)
  DVE : max (top-8) + max_index -> first occurrence of max u = argmin d2
Tail: one GPSIMD sub (d2min = f_sq - mx) + one ACT Sqrt over all 64 tiles,
then two contiguous DMAs out (partition-major; host transposes).

This container's walrus build only supports ONE sync wait per instruction
and rejects the InstISA encodings concourse emits for
EVENT_SEMAPHORE_RANGE_CLEAR and TENSOR_TENSOR_REDUCE ("ISA wrong length").
Workarounds: native per-sem clears (sem_clear patch), native-only compute
instructions, and a post-pass that hoists extra sync waits onto standalone
EventSemaphore instructions on the same engine.
"""

from contextlib import ExitStack

import numpy as np

import concourse.bass as bass
import concourse.mybir as mybir
import concourse.tile as tile
from concourse.bass_utils import run_bass_kernel_spmd


def _sem_clear_native(self, sems):
    """Replace the EVENT_SEMAPHORE_RANGE_CLEAR InstISA (rejected by this
    container's walrus with "ISA wrong length") with per-semaphore native
    sem-wr-imm EventSemaphore instructions."""
    nums = list(sems) if isinstance(sems, range) else [sems.num]
    if isinstance(sems, range):
        # only clear sems the program actually touched
        used = set()
        for f in self.bass.m.functions:
            for bb in f.blocks:
                for inst in bb.instructions:
                    si = inst.sync_info
                    if si is None:
                        continue
                    for w in si.on_wait:
                        used.add(w.id)
                    for u in si.on_update:
                        used.add(u.id)
        nums = [n for n in nums if n in used]
    last = None
    for n in nums:
        si = mybir.SyncInfo(
            on_wait=[],
            on_update=[mybir.SyncUpdate(
                sync_type="semaphore", id=n,
                update_mode="sem-wr-imm", update_value=0,
            )],
        )
        last = self.add_instruction(mybir.InstEventSemaphore(
            name=f"I-{self.bass.next_id()}", sync_info=si,
        ))
    return last


bass.BassGpSimd.sem_clear = _sem_clear_native


def split_multi_waits(nc):
    """This walrus allows a single sync wait per instruction; hoist all but
    the last wait of each instruction onto fresh same-engine EventSemaphore
    instructions placed immediately before it."""
    n_split = 0
    for f in nc.m.functions:
        for bb in f.blocks:
            new = []
            for inst in bb.instructions:
                si = inst.sync_info
                if si is not None and len(si.on_wait) > 1:
                    waits = list(si.on_wait)
                    for w in waits[:-1]:
                        new.append(mybir.InstEventSemaphore(
                            name=f"I-{nc.next_id()}",
                            engine=inst.engine,
                            sync_info=mybir.SyncInfo(on_wait=[w], on_update=[]),
                        ))
                        n_split += 1
                    inst.sync_info = mybir.SyncInfo(
                        on_wait=[waits[-1]], on_update=list(si.on_update),
                    )
                new.append(inst)
            bb.instructions = new
    return n_split


N, K, D = 65536, 1024, 256
NCORES = 8
NSHARD = N // NCORES          # 8192 rows per core
P = 128                       # partitions
NTILES = NSHARD // P          # 64 row-tiles per core
DCH = D // P                  # 2 contraction chunks
MMW = 512                     # matmul moving width (psum bank limit)
KCH = K // MMW
GROUP = 4                     # row-tiles per feature DMA
PS_BUFS = 4

MM_DT = mybir.dt.float16      # matmul operand dtype, 1 cycle/column
F32 = mybir.dt.float32


def build_program(split_waits=True):
    """Build the single-core Bass/Tile program (same program on all 8 cores)."""
    nc = bass.Bass("TRN2", target_bir_lowering=False, debug=False)

    fhi = nc.dram_tensor("fhi", (DCH, P, NSHARD), MM_DT, kind="ExternalInput").ap()
    flo = nc.dram_tensor("flo", (DCH, P, NSHARD), MM_DT, kind="ExternalInput").ap()
    muhi = nc.dram_tensor("muhi", (DCH, P, K), MM_DT, kind="ExternalInput").ap()
    mulo = nc.dram_tensor("mulo", (DCH, P, K), MM_DT, kind="ExternalInput").ap()
    csqb = nc.dram_tensor("csqb", (P, K), F32, kind="ExternalInput").ap()
    fsq = nc.dram_tensor("fsq", (NTILES, P), F32, kind="ExternalInput").ap()
    dist = nc.dram_tensor("dist", (P, NTILES), F32, kind="ExternalOutput").ap()
    idx = nc.dram_tensor("idx", (P, NTILES), mybir.dt.uint32, kind="ExternalOutput").ap()

    with tile.TileContext(nc) as tc, ExitStack() as ctx:
        const = ctx.enter_context(tc.tile_pool(name="const", bufs=1))
        ftp = ctx.enter_context(tc.tile_pool(name="ft", bufs=4))
        psp = ctx.enter_context(tc.tile_pool(name="ps", bufs=PS_BUFS, space="PSUM"))
        up = ctx.enter_context(tc.tile_pool(name="u", bufs=4))
        urp = ctx.enter_context(tc.tile_pool(name="ur", bufs=4))

        # Resident constants
        mh_sb = const.tile([P, DCH, K], MM_DT, tag="mh")
        nc.sync.dma_start(mh_sb[:], muhi.rearrange("d p k -> p d k"))
        ml_sb = const.tile([P, DCH, K], MM_DT, tag="ml")
        nc.sync.dma_start(ml_sb[:], mulo.rearrange("d p k -> p d k"))
        csq_sb = const.tile([P, K], F32, tag="csq")
        nc.sync.dma_start(csq_sb[:], csqb)
        fsq_sb = const.tile([P, NTILES], F32, tag="fsq")
        nc.sync.dma_start(fsq_sb[:], fsq.rearrange("t p -> p t"))
        dist_acc = const.tile([P, NTILES], F32, tag="dacc")
        d2min = const.tile([P, NTILES], F32, tag="d2min")
        mx_acc = const.tile([P, NTILES, 8], F32, tag="mxacc")
        idx_acc = const.tile([P, NTILES, 8], mybir.dt.uint32, tag="iacc")

        for g in range(NTILES // GROUP):
            gsl = slice(g * GROUP * P, (g + 1) * GROUP * P)
            fh_sb = ftp.tile([P, DCH, GROUP * P], MM_DT, tag="fh", name="fh_sb")
            nc.sync.dma_start(fh_sb[:], fhi[:, :, gsl].rearrange("d p n -> p d n"))
            fl_sb = ftp.tile([P, DCH, GROUP * P], MM_DT, tag="fl", name="fl_sb")
            nc.sync.dma_start(fl_sb[:], flo[:, :, gsl].rearrange("d p n -> p d n"))
            for j in range(GROUP):
                i = g * GROUP + j
                jsl = slice(j * P, (j + 1) * P)
                ps = psp.tile([P, K], F32)
                for kc in range(KCH):
                    sl = slice(kc * MMW, (kc + 1) * MMW)
                    mms = [
                        (fh_sb[:, 0, jsl], mh_sb[:, 0, sl]),
                        (fh_sb[:, 0, jsl], ml_sb[:, 0, sl]),
                        (fh_sb[:, 1, jsl], mh_sb[:, 1, sl]),
                        (fh_sb[:, 1, jsl], ml_sb[:, 1, sl]),
                        (fl_sb[:, 0, jsl], mh_sb[:, 0, sl]),
                        (fl_sb[:, 1, jsl], mh_sb[:, 1, sl]),
                    ]
                    for m, (lh, rh) in enumerate(mms):
                        nc.tensor.matmul(
                            ps[:, sl],
                            lhsT=lh,
                            rhs=rh,
                            start=(m == 0),
                            stop=(m == len(mms) - 1),
                        )
                ur = urp.tile([P, K], F32, tag="ur", name="ur")
                nc.scalar.copy(ur[:], ps[:])
                u = up.tile([P, K], F32)
                nc.gpsimd.tensor_sub(u[:], ur[:], csq_sb[:])
                nc.vector.max(mx_acc[:, i, :], u[:])
                nc.vector.max_index(idx_acc[:, i, :], mx_acc[:, i, :], u[:])

        # d2min = f_sq - max_k u, distance = sqrt(d2min): batched over all tiles
        nc.gpsimd.tensor_sub(d2min[:], fsq_sb[:], mx_acc[:, :, 0])
        nc.scalar.activation(dist_acc[:], d2min[:],
                             mybir.ActivationFunctionType.Sqrt)
        nc.sync.dma_start(dist, dist_acc[:])
        nc.sync.dma_start(idx, idx_acc[:, :, 0])

    if split_waits:
        split_multi_waits(nc)
    return nc


def _split16(x):
    hi = x.astype(np.float16)
    lo = (x - hi.astype(np.float32)).astype(np.float16)
    return hi, lo


def host_inputs(features, mu):
    """Host-side prep: transpose/shard features, fp16 hi/lo split, norms."""
    f = np.ascontiguousarray(features.reshape(N, D), dtype=np.float32)
    c = np.ascontiguousarray(mu.reshape(K, D), dtype=np.float32)
    fT = np.ascontiguousarray(f.T)                       # (D, N)
    muT2 = np.ascontiguousarray((2.0 * c).T)             # (D, K)
    csq = np.einsum("kd,kd->k", c, c).astype(np.float32)  # (K,)
    fsq = np.einsum("nd,nd->n", f, f).astype(np.float32)  # (N,)
    fT_hi, fT_lo = _split16(fT)
    mu_hi, mu_lo = _split16(muT2)
    csqb = np.ascontiguousarray(np.broadcast_to(csq, (P, K)))
    in_maps = []
    for core in range(NCORES):
        sl = slice(core * NSHARD, (core + 1) * NSHARD)
        in_maps.append({
            "fhi": np.ascontiguousarray(fT_hi[:, sl]).reshape(DCH, P, NSHARD),
            "flo": np.ascontiguousarray(fT_lo[:, sl]).reshape(DCH, P, NSHARD),
            "muhi": mu_hi.reshape(DCH, P, K),
            "mulo": mu_lo.reshape(DCH, P, K),
            "csqb": csqb,
            "fsq": np.ascontiguousarray(fsq[sl]).reshape(NTILES, P),
        })
    return in_maps


_PROGRAM = None


def _program():
    global _PROGRAM
    if _PROGRAM is None:
        _PROGRAM = build_program()
    return _PROGRAM


def kernel(features, mu, _trace=False):
    features = np.asarray(features)
    mu = np.asarray(mu)
    nc = _program()
    in_maps = host_inputs(features, mu)
    res = run_bass_kernel_spmd(nc, in_maps, list(range(NCORES)), trace=_trace)
    distance = np.concatenate(
        [r["dist"].T.reshape(NSHARD) for r in res.results])
    index = np.concatenate(
        [r["idx"].T.reshape(NSHARD).astype(np.int32) for r in res.results]
    )
    max_dist = np.max(distance).reshape(1).astype(np.float32)
    if _trace:
        return (distance, index, max_dist), res
    return (distance, index, max_dist)
